# revision 31
# baseline (speedup 1.0000x reference)
"""Fused transformer block (LN -> causal MHA -> residual -> LN -> SiLU MLP -> residual)
on 8 Trainium2 NeuronCores.

v2 design:
- Tensor-parallel over heads (2 heads/core) for QKV + attention.
- Attention scores computed TRANSPOSED (S^T[k,q]) so the post-softmax matrix is
  already in lhsT layout for the A@V matmul (no PE transposes of probabilities).
  Softmax denominator rides along as a ones-column appended to V.
- AllToAll of the raw per-head attention outputs (2MB) replaces a ReduceScatter
  of partial O-projections (16MB). O-projection happens after the exchange,
  token-local, with the full Wo resident in SBUF as fp8 (weights-only
  quantization; activations stay bf16).
- Token-parallel MLP (512 tokens/core, replicated weights). W1 and W2 are each
  streamed from HBM exactly once. U spills through DRAM (bf16).
- x streamed in bf16 for LN1; residual path f32. b2 folded into the residual on
  the host. LayerNorm affine params folded into adjacent projections.
- Emission order interleaves QKV(batch1) into attention(batch0) and the
  O-proj/LN2 chunk work into attention(batch1) to keep the PE fed.
"""

import sys
import os

for _p in ("/opt/trn_rl_repo", "/root/.axon_site/_ro/trn_rl_repo"):
    if os.path.isdir(_p) and _p not in sys.path:
        sys.path.insert(0, _p)
        break

import numpy as np
import ml_dtypes

import concourse.bass as bass
from concourse import bacc
import concourse.mybir as mybir
import concourse.tile as tile
from concourse.masks import make_identity
from concourse.bass_utils import run_bass_kernel_spmd

F32 = mybir.dt.float32
BF16 = mybir.dt.bfloat16
FP8 = mybir.dt.float8e4

P = 128          # partitions / head_dim / token tile
H = 2048         # hidden
KS = H // P      # 16 k-subtiles over hidden
HEADS = 16
HL = 2           # heads per core
NCORES = 8
B = 2
T = 2048
NTOK = B * T     # 4096
TPB = T          # tokens per batch
MID = 4 * H      # 8192
MMT = MID // P   # 64 m-tiles over mid dim
DQK = 2 * HL * P   # 512 rows of fused QK projection per core
DV = HL * P        # 256 V/attention-out features per core
EPS = 1e-5
NEG = -1.0e30

QT_PER_B = TPB // P   # 16 q tiles per batch
MT = NTOK // P        # 32 token m-tiles
NCHUNK = 4            # a2a chunks (1024 tokens each)
GT = 256              # tokens per A-group
NG_PER_B = TPB // GT  # 8 A-groups per batch


def build(sim=False, trn_kwargs=None, trace_sim=False):
    nc = bacc.Bacc(None, num_devices=NCORES, **(trn_kwargs or {}))

    x_d = nc.declare_dram_parameter("xbf", [NTOK, H], BF16, isOutput=False)
    xres_d = nc.declare_dram_parameter("xres", [NCHUNK * P, H], BF16, isOutput=False)
    wqk_d = nc.declare_dram_parameter("wqk", [P, KS, DQK], BF16, isOutput=False)
    bqk_d = nc.declare_dram_parameter("bqk", [P, DQK // P], F32, isOutput=False)
    wv_d = nc.declare_dram_parameter("wv", [P, KS, DV], BF16, isOutput=False)
    bvbc_d = nc.declare_dram_parameter("bvbc", [P, HL, P], F32, isOutput=False)
    wo_d = nc.declare_dram_parameter("wo", [P, KS, H], FP8, isOutput=False)
    w1_d = nc.declare_dram_parameter("w1", [MMT, P, KS, P], BF16, isOutput=False)
    b1_d = nc.declare_dram_parameter("b1", [P, MMT], F32, isOutput=False)
    w2_d = nc.declare_dram_parameter("w2", [MID, H], BF16, isOutput=False)
    cmaskT_d = nc.declare_dram_parameter("cmaskT", [P, P], F32, isOutput=False)
    out_d = nc.declare_dram_parameter("out", [NCHUNK * P, H], F32, isOutput=True)

    from contextlib import ExitStack
    with tile.TileContext(nc, trace_sim=trace_sim) as tc:
        with ExitStack() as stack:
            dram = stack.enter_context(tc.tile_pool(name="dram", bufs=1, space="DRAM"))
            const = stack.enter_context(tc.tile_pool(name="const", bufs=1))
            wbig = stack.enter_context(tc.tile_pool(name="wbig", bufs=1))
            # wqk (16KB/part, dead after QKV) aliases h2T (16KB, live from C on)
            p_ali = stack.enter_context(tc.tile_pool(name="ali16", bufs=1))
            p_x = stack.enter_context(tc.tile_pool(name="xin", bufs=2))
            p_ln = stack.enter_context(tc.tile_pool(name="lnsmall", bufs=3))
            p_h = stack.enter_context(tc.tile_pool(name="htok", bufs=2))
            p_hT = stack.enter_context(tc.tile_pool(name="hT", bufs=2))
            p_kv = stack.enter_context(tc.tile_pool(name="kvq", bufs=2))
            p_ex = stack.enter_context(tc.tile_pool(name="expT", bufs=2))
            p_ao = stack.enter_context(tc.tile_pool(name="aot", bufs=2))
            p_af = stack.enter_context(tc.tile_pool(name="attnf", bufs=1))
            p_x2 = stack.enter_context(tc.tile_pool(name="x2", bufs=1))
            p_w1 = stack.enter_context(tc.tile_pool(name="w1pool", bufs=3))
            p_w2 = stack.enter_context(tc.tile_pool(name="w2pool", bufs=4))
            p_us = stack.enter_context(tc.tile_pool(name="ustage", bufs=2))
            p_uk = stack.enter_context(tc.tile_pool(name="ukpool", bufs=4))
            p_ev = stack.enter_context(tc.tile_pool(name="evict", bufs=2))
            psA = stack.enter_context(tc.tile_pool(name="psA", bufs=8, space="PSUM"))

            # ---- internal DRAM ----
            aot_dram = dram.tile([NTOK, DV], BF16)
            a2a_dram = dram.tile([NTOK, DV], BF16)
            ut_dram = dram.tile([MID, NCHUNK * P], BF16)
            x2_dram = dram.tile([NCHUNK * P, H], F32)

            # ---- constants / weights in SBUF ----
            ident = const.tile([P, P], BF16)
            make_identity(nc, ident)
            epsb = const.tile([P, 1], F32)
            nc.vector.memset(epsb[:], EPS)
            cmaskT = const.tile([P, P], F32)
            nc.sync.dma_start(cmaskT[:], cmaskT_d[:, :])
            bqk_sb = const.tile([P, DQK // P], F32)
            nc.sync.dma_start(bqk_sb[:], bqk_d[:, :])
            bvbc_sb = const.tile([P, HL, P], F32)
            nc.sync.dma_start(bvbc_sb[:], bvbc_d[:, :, :])
            b1_sb = const.tile([P, MMT], F32)
            nc.sync.dma_start(b1_sb[:], b1_d[:, :])
            wqk_sb = p_ali.tile([P, KS, DQK], BF16, tag="ali16", name="wqk_sb")
            # split across queues: startup DMA bandwidth is per-ring limited
            nc.gpsimd.dma_start(out=wqk_sb[:, :KS // 2, :],
                                in_=wqk_d[:, :KS // 2, :])
            nc.scalar.dma_start(out=wqk_sb[:, KS // 2:, :],
                                in_=wqk_d[:, KS // 2:, :])
            # wo (4MB) is DMA'd later, during attention(b0)
            wv_sb = wbig.tile([P, KS, DV], BF16)
            nc.scalar.dma_start(out=wv_sb[:], in_=wv_d[:, :, :])
            wo_sb = wbig.tile([P, KS, H], FP8)
            # (-mean, rstd) for batch-1 LN1 tiles, precomputed before any Exp
            # lands on the scalar queue (avoids Sqrt<->Exp act-table thrash)
            lnP = const.tile([P, QT_PER_B, 2], F32)

            def layer_norm_stats(parts, name, nmu=None, rstd=None):
                """parts: list of (tile, ncols512) SBUF pieces, 4x512 cols total.
                Returns (nmu, rstd) [P,1] f32 APs; nmu = -mean. Destination APs
                may be passed in (e.g. slices of a persistent tile)."""
                st = p_ln.tile([P, 4, 6], F32, tag="lnst", name=f"st_{name}")
                a = 0
                for tile_, n in parts:
                    for i in range(n):
                        nc.vector.bn_stats(st[:, a, :], tile_[:, 512 * i:512 * (i + 1)])
                        a += 1
                assert a == 4
                mv = p_ln.tile([P, 2], F32, tag="lnmv", name=f"mv_{name}")
                nc.vector.bn_aggr(mv[:], st[:])
                sd = p_ln.tile([P, 1], F32, tag="lnsd", name=f"sd_{name}")
                nc.scalar.activation(sd[:], mv[:, 1:2],
                                     mybir.ActivationFunctionType.Sqrt, bias=epsb[:])
                if rstd is None:
                    rstd = p_ln.tile([P, 1], F32, tag="lnrstd", name=f"rstd_{name}")[:]
                nc.vector.reciprocal(rstd, sd[:])
                if nmu is None:
                    nmu = p_ln.tile([P, 1], F32, tag="lnnmu", name=f"nmu_{name}")[:]
                nc.vector.tensor_scalar_mul(nmu, mv[:, 0:1], -1.0)
                return nmu, rstd

            def ln_apply(dst, src, nmu, rstd, engine):
                """dst = (src - mean) * rstd via fused tensor_scalar."""
                engine.tensor_scalar(dst, src, nmu, rstd,
                                     mybir.AluOpType.add, mybir.AluOpType.mult)

            # ================= Stage A: LN1, transpose, QKV ===================
            ksb = [None, None]
            vsb = [None, None]
            qT = [None, None]

            def emit_A_group(b, g):
                """LN1 + transpose + QKV for GT=256 tokens (group g of batch b)."""
                if g == 0:
                    ksb[b] = p_kv.tile([P, HL, TPB], BF16, tag="ksb", name=f"ksb_{b}")
                    vsb[b] = p_kv.tile([P, QT_PER_B, HL, P + 2], BF16, tag="vsb",
                                       name=f"vsb_{b}")
                    qT[b] = p_kv.tile([P, HL, TPB], BF16, tag="qT", name=f"qT_{b}")
                    # ones columns for the softmax-denominator trick
                    nc.vector.memset(vsb[b][:, :, :, P:P + 1], 1.0)
                hT = p_hT.tile([P, KS, GT], BF16, tag="hT", name=f"hT_{b}_{g}")
                # pre-B window: vector is hot (stats), scalar idle -> psum
                # readers on scalar. B window: scalar owns Exp -> use vector.
                if b == 0:
                    ev_copy = lambda out, in_: nc.scalar.copy(out=out, in_=in_)
                    ev_bias = lambda out, in_, s: nc.scalar.add(out, in_, s)
                else:
                    ev_copy = lambda out, in_: nc.vector.tensor_copy(out=out, in_=in_)
                    ev_bias = lambda out, in_, s: nc.vector.tensor_scalar_add(
                        out, in_, s)
                for tt in range(GT // P):   # 128-token LN tiles
                    t = (TPB * b + GT * g) // P + tt
                    tl = (GT * g) // P + tt   # tile index within batch
                    xh = []
                    for hh in range(2):
                        xth = p_x.tile([P, H // 2], BF16, tag="xt",
                                       name=f"xt_{t}_{hh}")
                        (nc.sync if hh == 0 else nc.scalar).dma_start(
                            out=xth[:], in_=x_d[P * t:P * (t + 1),
                                               (H // 2) * hh:(H // 2) * (hh + 1)])
                        xh.append(xth)
                    if b == 1:
                        nmu, rstd = lnP[:, tl, 0:1], lnP[:, tl, 1:2]
                    else:
                        nmu, rstd = layer_norm_stats([(xh[0], 2), (xh[1], 2)],
                                                     f"ln1_{t}")
                    ht = p_h.tile([P, H], BF16, tag="ht", name=f"ht_{t}")
                    for hh in range(2):
                        ln_apply(ht[:, (H // 2) * hh:(H // 2) * (hh + 1)],
                                 xh[hh][:], nmu, rstd, nc.gpsimd)
                    for fg in range(KS // 8):
                        ptp = psA.tile([P, 1024], BF16, tag="psA", name=f"trp_{t}_{fg}")
                        for f4 in range(8):
                            f = 8 * fg + f4
                            nc.tensor.transpose(ptp[:, P * f4:P * (f4 + 1)],
                                                ht[:, P * f:P * (f + 1)], ident[:])
                        ev_copy(hT[:, 8 * fg:8 * (fg + 1), P * tt:P * (tt + 1)],
                                ptp[:].rearrange("p (a b) -> p a b", b=P))

                col0 = GT * g
                # QK projection: m 0,1 -> Q head0/1 ; 2,3 -> K head0/1
                for m in range(4):
                    ps = psA.tile([P, GT], F32, tag="psA", name=f"qk_{b}_{g}_{m}")
                    for ks in range(KS):
                        nc.tensor.matmul(ps[:], lhsT=wqk_sb[:, ks, P * m:P * (m + 1)],
                                         rhs=hT[:, ks, :],
                                         start=(ks == 0), stop=(ks == KS - 1))
                    dst = qT[b] if m < 2 else ksb[b]
                    ev_bias(dst[:, m % 2, col0:col0 + GT], ps[:],
                            bqk_sb[:, m:m + 1])
                # V projection (token-major)
                for m in range(GT // P):
                    ps = psA.tile([P, 512], F32, tag="psA", name=f"v_{b}_{g}_{m}")
                    for ks in range(KS):
                        nc.tensor.matmul(ps[:, :DV], lhsT=hT[:, ks, P * m:P * (m + 1)],
                                         rhs=wv_sb[:, ks, :],
                                         start=(ks == 0), stop=(ks == KS - 1))
                    tm = (GT * g) // P + m
                    nc.vector.tensor_tensor(
                        vsb[b][:, tm, :, 0:P],
                        ps[:, :DV].rearrange("p (a b) -> p a b", b=P),
                        bvbc_sb[:], mybir.AluOpType.add)

            # ================= Stage B: attention (S^T form) ==================
            aosb = {}

            def emit_B_S(b, qt, lh):
                """S^T matmuls + mask + exp for (batch, query tile, local head)."""
                klen = P * (qt + 1)
                nchs = (qt + 4) // 4
                ex = p_ex.tile([P, TPB], BF16, tag="ex", name=f"ex_{b}_{qt}_{lh}")
                qcols = qT[b][:, lh, P * qt:P * (qt + 1)]
                for j in range(nchs):
                    n0 = 512 * j
                    n1 = min(n0 + 512, klen)
                    ps = psA.tile([P, 512], F32, tag="psA", name=f"s_{b}_{qt}_{lh}_{j}")
                    for kb in range(n0 // P, n1 // P):
                        nc.tensor.matmul(ps[:, P * kb - n0:P * (kb + 1) - n0],
                                         lhsT=ksb[b][:, lh, P * kb:P * (kb + 1)],
                                         rhs=qcols, start=True, stop=True)
                    if j == nchs - 1:
                        d0 = klen - P - n0
                        nc.vector.tensor_tensor(ps[:, d0:d0 + P], ps[:, d0:d0 + P],
                                                cmaskT[:], mybir.AluOpType.add)
                    nc.scalar.activation(ex[:, n0:n1], ps[:, :n1 - n0],
                                         mybir.ActivationFunctionType.Exp)
                return ex

            def emit_B_AV(b, qt, lh, ex):
                """A@V with ones-column, normalize, stage aot; DMA after lh=1."""
                mt = QT_PER_B * b + qt
                if lh == 0:
                    aosb[mt] = p_ao.tile([P, HL, P], BF16, tag="aot", name=f"ao_{mt}")
                psO = psA.tile([P, P + 2], F32, tag="psA", name=f"o_{mt}_{lh}")
                for kb in range(qt + 1):
                    nc.tensor.matmul(psO[:, :P + 1],
                                     lhsT=ex[:, P * kb:P * (kb + 1)],
                                     rhs=vsb[b][:, kb, lh, 0:P + 1],
                                     start=(kb == 0), stop=(kb == qt))
                rinv = p_ln.tile([P, 1], F32, tag="rinv", name=f"ri_{mt}_{lh}")
                nc.vector.reciprocal(rinv[:], psO[:, P:P + 1])
                nc.vector.tensor_scalar_mul(aosb[mt][:, lh, :], psO[:, 0:P], rinv[:])
                if lh == HL - 1:
                    nc.sync.dma_start(aot_dram[P * mt:P * (mt + 1), :],
                                      aosb[mt][:].rearrange("p a b -> p (a b)"))
                    del aosb[mt]

            rg = [list(range(NCORES))]

            def emit_collective(j):
                nc.gpsimd.collective_compute(
                    "AllToAll", mybir.AluOpType.bypass, replica_groups=rg,
                    ins=[aot_dram[1024 * j:1024 * (j + 1), :]],
                    outs=[a2a_dram[1024 * j:1024 * (j + 1), :]])

            # ================= Stage C: O-proj + LN2 per chunk ================
            h2T = p_ali.tile([P, KS, NCHUNK * P], BF16, tag="ali16", name="h2T")

            def emit_C(j):
                af = p_af.tile([P, H], BF16, tag="af", name=f"af_{j}")
                nc.sync.dma_start(
                    af[:].rearrange("p (s f) -> p s f", f=DV),
                    a2a_dram[1024 * j:1024 * (j + 1), :]
                    .rearrange("(s p) f -> p s f", p=P))
                # transpose attn_full -> attnT [feat, tok]
                afT = p_af.tile([P, KS, P], BF16, tag="afT", name=f"afT_{j}")
                for fg in range(2):
                    ptp = psA.tile([P, 1024], BF16, tag="psA", name=f"at_{j}_{fg}")
                    for f4 in range(8):
                        f = 8 * fg + f4
                        nc.tensor.transpose(ptp[:, P * f4:P * (f4 + 1)],
                                            af[:, P * f:P * (f + 1)], ident[:])
                    nc.vector.tensor_copy(
                        out=afT[:, 8 * fg:8 * (fg + 1), :],
                        in_=ptp[:].rearrange("p (a b) -> p a b", b=P))
                # O-projection (2 passes of 2 psum banks) + residual
                x2 = p_x2.tile([P, H], F32, tag="x2", name=f"x2_{j}")
                xr = p_x2.tile([P, H], BF16, tag="xr", name=f"xr_{j}")
                nc.sync.dma_start(xr[:], xres_d[P * j:P * (j + 1), :])
                for half in range(2):
                    pss = []
                    for nn in range(2):
                        psn = psA.tile([P, 512], F32, tag="psA",
                                       name=f"op_{j}_{half}_{nn}")
                        pss.append(psn)
                    for kb in range(KS):
                        for nn in range(2):
                            nk = 2 * half + nn
                            nc.tensor.matmul(
                                pss[nn][:], lhsT=afT[:, kb, :],
                                rhs=wo_sb[:, kb, 512 * nk:512 * (nk + 1)],
                                start=(kb == 0), stop=(kb == KS - 1))
                    for nn in range(2):
                        c0 = 1024 * half + 512 * nn
                        nc.vector.tensor_tensor(x2[:, c0:c0 + 512], pss[nn][:],
                                                xr[:, c0:c0 + 512],
                                                mybir.AluOpType.add)
                nc.sync.dma_start(x2_dram[P * j:P * (j + 1), :], x2[:])
                # LN2 -> h2 -> h2T
                nmu, rstd = layer_norm_stats([(x2, 4)], f"ln2_{j}")
                h2 = p_h.tile([P, H], BF16, tag="ht", name=f"h2_{j}")
                for hh in range(2):
                    ln_apply(h2[:, (H // 2) * hh:(H // 2) * (hh + 1)],
                             x2[:, (H // 2) * hh:(H // 2) * (hh + 1)],
                             nmu, rstd, nc.gpsimd)
                for fg in range(KS // 8):
                    ptp = psA.tile([P, 1024], BF16, tag="psA", name=f"h2t_{j}_{fg}")
                    for f4 in range(8):
                        f = 8 * fg + f4
                        nc.tensor.transpose(ptp[:, P * f4:P * (f4 + 1)],
                                            h2[:, P * f:P * (f + 1)], ident[:])
                    nc.vector.tensor_copy(
                        out=h2T[:, 8 * fg:8 * (fg + 1), P * j:P * (j + 1)],
                        in_=ptp[:].rearrange("p (a b) -> p a b", b=P))

            def emit_A0_stats(g):
                """Pre-compute batch-1 LN1 stats for group g into lnP."""
                for tt in range(GT // P):
                    t = (TPB + GT * g) // P + tt
                    tl = (GT * g) // P + tt
                    xh = []
                    for hh in range(2):
                        xth = p_x.tile([P, H // 2], BF16, tag="x0",
                                       name=f"x0_{t}_{hh}")
                        (nc.sync if hh == 0 else nc.gpsimd).dma_start(
                            out=xth[:], in_=x_d[P * t:P * (t + 1),
                                               (H // 2) * hh:(H // 2) * (hh + 1)])
                        xh.append(xth)
                    layer_norm_stats([(xh[0], 2), (xh[1], 2)], f"ln0_{t}",
                                     nmu=lnP[:, tl, 0:1], rstd=lnP[:, tl, 1:2])

            # ================= emission schedule ==============================
            for g in range(NG_PER_B):
                emit_A_group(0, g)
                emit_A0_stats(g)

            # attention(b0) interleaved with QKV(b1)
            for qt in range(QT_PER_B):
                exs = [emit_B_S(0, qt, lh) for lh in range(HL)]
                for lh in range(HL):
                    emit_B_AV(0, qt, lh, exs[lh])
                if qt % 2 == 1:
                    emit_A_group(1, qt // 2)
                if qt == 1:
                    nc.scalar.dma_start(out=wo_sb[:], in_=wo_d[:, :, :])
                if qt == 7:
                    emit_collective(0)
            emit_collective(1)

            # attention(b1) interleaved with chunk 0/1 post-processing
            for qt in range(QT_PER_B):
                exs = [emit_B_S(1, qt, lh) for lh in range(HL)]
                for lh in range(HL):
                    emit_B_AV(1, qt, lh, exs[lh])
                if qt == 3:
                    emit_C(0)
                if qt == 11:
                    emit_C(1)
                if qt == 7:
                    emit_collective(2)
            emit_collective(3)
            emit_C(2)
            emit_C(3)

            # ================= Stage D: MLP1 ==================================
            silu_fn = (mybir.ActivationFunctionType.Sigmoid if sim
                       else mybir.ActivationFunctionType.Silu)
            for mm in range(MMT):
                w1t = p_w1.tile([P, KS, P], BF16, tag="w1t", name=f"w1t_{mm}")
                nc.gpsimd.dma_start(out=w1t[:], in_=w1_d[mm, :, :, :])
                ps = psA.tile([P, 512], F32, tag="psA", name=f"u_{mm}")
                for ks in range(KS):
                    nc.tensor.matmul(ps[:],
                                     lhsT=w1t[:, ks, :],
                                     rhs=h2T[:, ks, :],
                                     start=(ks == 0), stop=(ks == KS - 1))
                usl = p_us.tile([P, 512], BF16, tag="usl", name=f"usl_{mm}")
                nc.scalar.activation(usl[:], ps[:], silu_fn,
                                     bias=b1_sb[:, mm:mm + 1])
                nc.sync.dma_start(ut_dram[P * mm:P * (mm + 1), :], usl[:])

            # ================= Stage E: MLP2 ==================================
            for ng in range(2):
                pss = [[None] * 2 for _ in range(4)]
                for jj in range(4):
                    for nn in range(2):
                        pss[jj][nn] = psA.tile([P, 512], F32, tag="psA",
                                               name=f"y_{ng}_{jj}_{nn}")
                x2sls = [[None] * 2 for _ in range(4)]

                def emit_x2sl(ng, jj):
                    for nn in range(2):
                        c0 = 1024 * ng + 512 * nn
                        x2sls[jj][nn] = p_ev.tile([P, 512], F32, tag="x2sl",
                                                  name=f"x2sl_{ng}_{jj}_{nn}",
                                                  bufs=4)
                        nc.scalar.dma_start(
                            out=x2sls[jj][nn][:],
                            in_=x2_dram[P * jj:P * (jj + 1), c0:c0 + 512])

                for mm in range(MMT):
                    utk = p_uk.tile([P, 512], BF16, tag="utk", name=f"utk_{ng}_{mm}")
                    nc.sync.dma_start(utk[:], ut_dram[P * mm:P * (mm + 1), :])
                    w2t = p_w2.tile([P, 1024], BF16, tag="w2t", name=f"w2t_{ng}_{mm}")
                    nc.gpsimd.dma_start(
                        out=w2t[:], in_=w2_d[P * mm:P * (mm + 1),
                                             1024 * ng:1024 * (ng + 1)])
                    for jj in range(4):
                        for nn in range(2):
                            nc.tensor.matmul(
                                pss[jj][nn][:],
                                lhsT=utk[:, P * jj:P * (jj + 1)],
                                rhs=w2t[:, 512 * nn:512 * (nn + 1)],
                                start=(mm == 0), stop=(mm == MMT - 1))
                    # prefetch the residual slices into SBUF before the drain
                    if mm == MMT - 4:
                        emit_x2sl(ng, 0)
                        emit_x2sl(ng, 1)
                # drain: adds on vector, DMAs on the idle scalar queue so the
                # next pass's utk/w2t prefetch flows on sync/gpsimd
                for jj in range(4):
                    if jj == 2:
                        emit_x2sl(ng, 2)
                        emit_x2sl(ng, 3)
                    for nn in range(2):
                        c0 = 1024 * ng + 512 * nn
                        ot = p_ev.tile([P, 512], F32, tag="ot",
                                       name=f"ot_{ng}_{jj}_{nn}")
                        nc.vector.tensor_tensor(ot[:], pss[jj][nn][:],
                                                x2sls[jj][nn][:],
                                                mybir.AluOpType.add)
                        nc.scalar.dma_start(
                            out=out_d[P * jj:P * (jj + 1), c0:c0 + 512], in_=ot[:])
    nc.compile()
    return nc


def _bf16(a):
    return np.asarray(a, dtype=np.float32).astype(ml_dtypes.bfloat16)


def make_in_maps(x, Wq, Wk, Wv, Wo, g1, bn1, g2, bn2, W1, b1, W2, b2):
    x = np.asarray(x, np.float32)
    x_flat = np.ascontiguousarray(x.reshape(NTOK, H))
    s = np.float32(1.0 / np.sqrt(P))
    fp8_np = mybir.dt.np(FP8)

    wq_eff = (g1[:, None] * np.asarray(Wq, np.float32)) * s
    wk_eff = g1[:, None] * np.asarray(Wk, np.float32)
    wv_eff = g1[:, None] * np.asarray(Wv, np.float32)
    bq = (bn1 @ np.asarray(Wq, np.float32)) * s
    bk = bn1 @ np.asarray(Wk, np.float32)
    bv = bn1 @ np.asarray(Wv, np.float32)
    w1_eff = g2[:, None] * np.asarray(W1, np.float32)
    b1_eff = np.asarray(b1, np.float32) + bn2 @ np.asarray(W1, np.float32)

    # shared tensors
    xbf = np.ascontiguousarray(_bf16(x_flat))
    w1_t = np.ascontiguousarray(
        _bf16(w1_eff).reshape(KS, P, MMT, P).transpose(2, 1, 0, 3))  # [mm, p, ks, mw]
    w2_t = np.ascontiguousarray(_bf16(W2))
    b1m = np.ascontiguousarray(b1_eff.reshape(MMT, P).T.astype(np.float32))
    wo8 = np.ascontiguousarray(
        np.asarray(Wo, np.float32).astype(fp8_np).reshape(KS, P, H).transpose(1, 0, 2))
    ii, jj_ = np.meshgrid(np.arange(P), np.arange(P), indexing="ij")
    cmaskT = np.where(ii <= jj_, 0.0, NEG).astype(np.float32)
    b2f = np.asarray(b2, np.float32)

    in_maps = []
    for c in range(NCORES):
        cs = slice(DV * c, DV * (c + 1))
        wqk = np.concatenate([wq_eff[:, cs], wk_eff[:, cs]], axis=1)  # [H, 512]
        wqk_t = np.ascontiguousarray(
            _bf16(wqk).reshape(KS, P, DQK).transpose(1, 0, 2))
        bqk = np.concatenate([bq[cs], bk[cs]]).astype(np.float32)
        bqk_m = np.ascontiguousarray(bqk.reshape(DQK // P, P).T)
        wv_t = np.ascontiguousarray(
            _bf16(wv_eff[:, cs]).reshape(KS, P, DV).transpose(1, 0, 2))
        bvbc = np.ascontiguousarray(np.broadcast_to(
            bv[cs].astype(np.float32).reshape(1, HL, P), (P, HL, P)))
        xres = np.concatenate(
            [x_flat[1024 * j + P * c:1024 * j + P * (c + 1)] for j in range(NCHUNK)],
            axis=0) + b2f
        in_maps.append({
            "xbf": xbf, "xres": np.ascontiguousarray(_bf16(xres)),
            "wqk": wqk_t, "bqk": bqk_m, "wv": wv_t, "bvbc": bvbc, "wo": wo8,
            "w1": w1_t, "b1": b1m, "w2": w2_t, "cmaskT": cmaskT,
        })
    return in_maps


_NC_CACHE = {}


def kernel(**inputs):
    if "nc" not in _NC_CACHE:
        _NC_CACHE["nc"] = build()
    nc = _NC_CACHE["nc"]
    in_maps = make_in_maps(
        inputs["x"], inputs["Wq"], inputs["Wk"], inputs["Wv"], inputs["Wo"],
        np.asarray(inputs["g1"], np.float32), np.asarray(inputs["bn1"], np.float32),
        np.asarray(inputs["g2"], np.float32), np.asarray(inputs["bn2"], np.float32),
        inputs["W1"], inputs["b1"], inputs["W2"], inputs["b2"])
    res = run_bass_kernel_spmd(nc, in_maps, list(range(NCORES)))
    out = np.empty((NTOK, H), np.float32)
    for c in range(NCORES):
        oc = res.results[c]["out"]
        for j in range(NCHUNK):
            out[1024 * j + P * c:1024 * j + P * (c + 1)] = oc[P * j:P * (j + 1)]
    return out.reshape(B, T, H)


# revision 35
# speedup vs baseline: 1.0765x; 1.0765x over previous
"""Fused transformer block (LN -> causal MHA -> residual -> LN -> SiLU MLP -> residual)
on 8 Trainium2 NeuronCores.

v2 design:
- Tensor-parallel over heads (2 heads/core) for QKV + attention.
- Attention scores computed TRANSPOSED (S^T[k,q]) so the post-softmax matrix is
  already in lhsT layout for the A@V matmul (no PE transposes of probabilities).
  Softmax denominator rides along as a ones-column appended to V.
- AllToAll of the raw per-head attention outputs (2MB) replaces a ReduceScatter
  of partial O-projections (16MB). O-projection happens after the exchange,
  token-local, with the full Wo resident in SBUF as fp8 (weights-only
  quantization; activations stay bf16).
- Token-parallel MLP (512 tokens/core, replicated weights). W1 and W2 are each
  streamed from HBM exactly once. U spills through DRAM (bf16).
- x streamed in bf16 for LN1; residual path f32. b2 folded into the residual on
  the host. LayerNorm affine params folded into adjacent projections.
- Emission order interleaves QKV(batch1) into attention(batch0) and the
  O-proj/LN2 chunk work into attention(batch1) to keep the PE fed.
"""

import sys
import os

for _p in ("/opt/trn_rl_repo", "/root/.axon_site/_ro/trn_rl_repo"):
    if os.path.isdir(_p) and _p not in sys.path:
        sys.path.insert(0, _p)
        break

import numpy as np
import ml_dtypes

import concourse.bass as bass
from concourse import bacc
import concourse.mybir as mybir
import concourse.tile as tile
from concourse.masks import make_identity
from concourse.bass_utils import run_bass_kernel_spmd

F32 = mybir.dt.float32
BF16 = mybir.dt.bfloat16
FP8 = mybir.dt.float8e4

P = 128          # partitions / head_dim / token tile
H = 2048         # hidden
KS = H // P      # 16 k-subtiles over hidden
HEADS = 16
HL = 2           # heads per core
NCORES = 8
B = 2
T = 2048
NTOK = B * T     # 4096
TPB = T          # tokens per batch
MID = 4 * H      # 8192
MMT = MID // P   # 64 m-tiles over mid dim
DQK = 2 * HL * P   # 512 rows of fused QK projection per core
DV = HL * P        # 256 V/attention-out features per core
EPS = 1e-5
NEG = -1.0e30

QT_PER_B = TPB // P   # 16 q tiles per batch
MT = NTOK // P        # 32 token m-tiles
NCHUNK = 4            # a2a chunks (1024 tokens each)
GT = 256              # tokens per A-group
NG_PER_B = TPB // GT  # 8 A-groups per batch


def build(sim=False, trn_kwargs=None, trace_sim=False):
    nc = bacc.Bacc(None, num_devices=NCORES, **(trn_kwargs or {}))

    x_d = nc.declare_dram_parameter("xbf", [NTOK, H], BF16, isOutput=False)
    xres_d = nc.declare_dram_parameter("xres", [NCHUNK * P, H], BF16, isOutput=False)
    wqk_d = nc.declare_dram_parameter("wqk", [P, KS, DQK], BF16, isOutput=False)
    bqk_d = nc.declare_dram_parameter("bqk", [P, DQK // P], F32, isOutput=False)
    wv_d = nc.declare_dram_parameter("wv", [P, KS, DV], BF16, isOutput=False)
    bvbc_d = nc.declare_dram_parameter("bvbc", [P, HL, P], F32, isOutput=False)
    wo_d = nc.declare_dram_parameter("wo", [P, KS, H], FP8, isOutput=False)
    w1_d = nc.declare_dram_parameter("w1", [MMT, P, KS, P], BF16, isOutput=False)
    b1_d = nc.declare_dram_parameter("b1", [P, MMT], F32, isOutput=False)
    w2_d = nc.declare_dram_parameter("w2", [MID, H], BF16, isOutput=False)
    cmaskT_d = nc.declare_dram_parameter("cmaskT", [P, P], F32, isOutput=False)
    out_d = nc.declare_dram_parameter("out", [NCHUNK * P, H], F32, isOutput=True)

    from contextlib import ExitStack
    with tile.TileContext(nc, trace_sim=trace_sim) as tc:
        with ExitStack() as stack:
            dram = stack.enter_context(tc.tile_pool(name="dram", bufs=1, space="DRAM"))
            const = stack.enter_context(tc.tile_pool(name="const", bufs=1))
            wbig = stack.enter_context(tc.tile_pool(name="wbig", bufs=1))
            # wqk (16KB/part, dead after QKV) aliases h2T (16KB, live from C on)
            p_ali = stack.enter_context(tc.tile_pool(name="ali16", bufs=1))
            p_x = stack.enter_context(tc.tile_pool(name="xin", bufs=2))
            p_ln = stack.enter_context(tc.tile_pool(name="lnsmall", bufs=3))
            p_h = stack.enter_context(tc.tile_pool(name="htok", bufs=2))
            p_hT = stack.enter_context(tc.tile_pool(name="hT", bufs=2))
            p_kv = stack.enter_context(tc.tile_pool(name="kvq", bufs=2))
            p_ex = stack.enter_context(tc.tile_pool(name="expT", bufs=2))
            p_ao = stack.enter_context(tc.tile_pool(name="aot", bufs=2))
            p_af = stack.enter_context(tc.tile_pool(name="attnf", bufs=1))
            p_x2 = stack.enter_context(tc.tile_pool(name="x2", bufs=1))
            p_w1 = stack.enter_context(tc.tile_pool(name="w1pool", bufs=3))
            p_w2 = stack.enter_context(tc.tile_pool(name="w2pool", bufs=4))
            p_us = stack.enter_context(tc.tile_pool(name="ustage", bufs=2))
            p_uk = stack.enter_context(tc.tile_pool(name="ukpool", bufs=4))
            p_ev = stack.enter_context(tc.tile_pool(name="evict", bufs=2))
            psA = stack.enter_context(tc.tile_pool(name="psA", bufs=8, space="PSUM"))

            # ---- internal DRAM ----
            aot_dram = dram.tile([NTOK, DV], BF16)
            a2a_dram = dram.tile([NTOK, DV], BF16)
            ut_dram = dram.tile([MID, NCHUNK * P], BF16)
            x2_dram = dram.tile([NCHUNK * P, H], F32)

            # ---- constants / weights in SBUF ----
            ident = const.tile([P, P], BF16)
            make_identity(nc, ident)
            epsb = const.tile([P, 1], F32)
            nc.vector.memset(epsb[:], EPS)
            cmaskT = const.tile([P, P], F32)
            nc.sync.dma_start(cmaskT[:], cmaskT_d[:, :])
            bqk_sb = const.tile([P, DQK // P], F32)
            nc.sync.dma_start(bqk_sb[:], bqk_d[:, :])
            bvbc_sb = const.tile([P, HL, P], F32)
            nc.sync.dma_start(bvbc_sb[:], bvbc_d[:, :, :])
            b1_sb = const.tile([P, MMT], F32)
            nc.sync.dma_start(b1_sb[:], b1_d[:, :])
            wqk_sb = p_ali.tile([P, KS, DQK], BF16, tag="ali16", name="wqk_sb")
            # split across queues: startup DMA bandwidth is per-ring limited
            nc.gpsimd.dma_start(out=wqk_sb[:, :KS // 2, :],
                                in_=wqk_d[:, :KS // 2, :])
            nc.scalar.dma_start(out=wqk_sb[:, KS // 2:, :],
                                in_=wqk_d[:, KS // 2:, :])
            # wo (4MB) is DMA'd later, during attention(b0)
            wv_sb = wbig.tile([P, KS, DV], BF16)
            nc.scalar.dma_start(out=wv_sb[:], in_=wv_d[:, :, :])
            wo_sb = wbig.tile([P, KS, H], FP8)

            def layer_norm_stats(parts, name):
                """parts: list of (tile, ncols512) SBUF pieces, 4x512 cols total.
                Returns (nmu, rstd) [P,1] f32 APs; nmu = -mean.
                rstd = exp(-0.5*ln(var+eps)): Ln and Exp live in the SAME
                activation table set, so this never forces a table reload
                against the attention Exps (unlike Sqrt)."""
                st = p_ln.tile([P, 4, 6], F32, tag="lnst", name=f"st_{name}")
                a = 0
                for tile_, n in parts:
                    for i in range(n):
                        nc.vector.bn_stats(st[:, a, :], tile_[:, 512 * i:512 * (i + 1)])
                        a += 1
                assert a == 4
                mv = p_ln.tile([P, 2], F32, tag="lnmv", name=f"mv_{name}")
                nc.vector.bn_aggr(mv[:], st[:])
                lv = p_ln.tile([P, 1], F32, tag="lnsd", name=f"lv_{name}")
                nc.scalar.activation(lv[:], mv[:, 1:2],
                                     mybir.ActivationFunctionType.Ln, bias=epsb[:])
                rstd = p_ln.tile([P, 1], F32, tag="lnrstd", name=f"rstd_{name}")
                nc.scalar.activation(rstd[:], lv[:],
                                     mybir.ActivationFunctionType.Exp, scale=-0.5)
                nmu = p_ln.tile([P, 1], F32, tag="lnnmu", name=f"nmu_{name}")
                nc.vector.tensor_scalar_mul(nmu[:], mv[:, 0:1], -1.0)
                return nmu[:], rstd[:]

            def ln_apply(dst, src, nmu, rstd, engine):
                """dst = (src - mean) * rstd via fused tensor_scalar."""
                engine.tensor_scalar(dst, src, nmu, rstd,
                                     mybir.AluOpType.add, mybir.AluOpType.mult)

            # ================= Stage A: LN1, transpose, QKV ===================
            ksb = [None, None]
            vsb = [None, None]
            qT = [None, None]

            def emit_A_group(b, g):
                """LN1 + transpose + QKV for GT=256 tokens (group g of batch b)."""
                if g == 0:
                    ksb[b] = p_kv.tile([P, HL, TPB], BF16, tag="ksb", name=f"ksb_{b}")
                    vsb[b] = p_kv.tile([P, QT_PER_B, HL, P + 2], BF16, tag="vsb",
                                       name=f"vsb_{b}")
                    qT[b] = p_kv.tile([P, HL, TPB], BF16, tag="qT", name=f"qT_{b}")
                    # ones columns for the softmax-denominator trick
                    nc.vector.memset(vsb[b][:, :, :, P:P + 1], 1.0)
                hT = p_hT.tile([P, KS, GT], BF16, tag="hT", name=f"hT_{b}_{g}")
                # pre-B window: vector is hot (stats), scalar idle -> psum
                # readers on scalar. B window: scalar owns Exp -> use vector.
                if b == 0:
                    ev_copy = lambda out, in_: nc.scalar.copy(out=out, in_=in_)
                    ev_bias = lambda out, in_, s: nc.scalar.add(out, in_, s)
                else:
                    ev_copy = lambda out, in_: nc.vector.tensor_copy(out=out, in_=in_)
                    ev_bias = lambda out, in_, s: nc.vector.tensor_scalar_add(
                        out, in_, s)
                for tt in range(GT // P):   # 128-token LN tiles
                    t = (TPB * b + GT * g) // P + tt
                    xh = []
                    for hh in range(2):
                        xth = p_x.tile([P, H // 2], BF16, tag="xt",
                                       name=f"xt_{t}_{hh}", bufs=4)
                        (nc.sync if hh == 0 else nc.scalar).dma_start(
                            out=xth[:], in_=x_d[P * t:P * (t + 1),
                                               (H // 2) * hh:(H // 2) * (hh + 1)])
                        xh.append(xth)
                    nmu, rstd = layer_norm_stats([(xh[0], 2), (xh[1], 2)],
                                                 f"ln1_{t}")
                    ht = p_h.tile([P, H], BF16, tag="ht", name=f"ht_{t}")
                    for hh in range(2):
                        ln_apply(ht[:, (H // 2) * hh:(H // 2) * (hh + 1)],
                                 xh[hh][:], nmu, rstd, nc.gpsimd)
                    for fg in range(KS // 8):
                        ptp = psA.tile([P, 1024], BF16, tag="psA", name=f"trp_{t}_{fg}")
                        for f4 in range(8):
                            f = 8 * fg + f4
                            nc.tensor.transpose(ptp[:, P * f4:P * (f4 + 1)],
                                                ht[:, P * f:P * (f + 1)], ident[:])
                        ev_copy(hT[:, 8 * fg:8 * (fg + 1), P * tt:P * (tt + 1)],
                                ptp[:].rearrange("p (a b) -> p a b", b=P))

                col0 = GT * g
                # QK projection: m 0,1 -> Q head0/1 ; 2,3 -> K head0/1
                for m in range(4):
                    ps = psA.tile([P, GT], F32, tag="psA", name=f"qk_{b}_{g}_{m}")
                    for ks in range(KS):
                        nc.tensor.matmul(ps[:], lhsT=wqk_sb[:, ks, P * m:P * (m + 1)],
                                         rhs=hT[:, ks, :],
                                         start=(ks == 0), stop=(ks == KS - 1))
                    dst = qT[b] if m < 2 else ksb[b]
                    ev_bias(dst[:, m % 2, col0:col0 + GT], ps[:],
                            bqk_sb[:, m:m + 1])
                # V projection (token-major)
                for m in range(GT // P):
                    ps = psA.tile([P, 512], F32, tag="psA", name=f"v_{b}_{g}_{m}")
                    for ks in range(KS):
                        nc.tensor.matmul(ps[:, :DV], lhsT=hT[:, ks, P * m:P * (m + 1)],
                                         rhs=wv_sb[:, ks, :],
                                         start=(ks == 0), stop=(ks == KS - 1))
                    tm = (GT * g) // P + m
                    nc.vector.tensor_tensor(
                        vsb[b][:, tm, :, 0:P],
                        ps[:, :DV].rearrange("p (a b) -> p a b", b=P),
                        bvbc_sb[:], mybir.AluOpType.add)

            # ================= Stage B: attention (S^T form) ==================
            aosb = {}

            def emit_B_S(b, qt, lh):
                """S^T matmuls + mask + exp for (batch, query tile, local head)."""
                klen = P * (qt + 1)
                nchs = (qt + 4) // 4
                ex = p_ex.tile([P, TPB], BF16, tag="ex", name=f"ex_{b}_{qt}_{lh}")
                qcols = qT[b][:, lh, P * qt:P * (qt + 1)]
                for j in range(nchs):
                    n0 = 512 * j
                    n1 = min(n0 + 512, klen)
                    ps = psA.tile([P, 512], F32, tag="psA", name=f"s_{b}_{qt}_{lh}_{j}")
                    for kb in range(n0 // P, n1 // P):
                        nc.tensor.matmul(ps[:, P * kb - n0:P * (kb + 1) - n0],
                                         lhsT=ksb[b][:, lh, P * kb:P * (kb + 1)],
                                         rhs=qcols, start=True, stop=True)
                    if j == nchs - 1:
                        d0 = klen - P - n0
                        nc.vector.tensor_tensor(ps[:, d0:d0 + P], ps[:, d0:d0 + P],
                                                cmaskT[:], mybir.AluOpType.add)
                    nc.scalar.activation(ex[:, n0:n1], ps[:, :n1 - n0],
                                         mybir.ActivationFunctionType.Exp)
                return ex

            def emit_B_AV(b, qt, lh, ex):
                """A@V with ones-column, normalize, stage aot; DMA after lh=1."""
                mt = QT_PER_B * b + qt
                if lh == 0:
                    aosb[mt] = p_ao.tile([P, HL, P], BF16, tag="aot", name=f"ao_{mt}")
                psO = psA.tile([P, P + 2], F32, tag="psA", name=f"o_{mt}_{lh}")
                for kb in range(qt + 1):
                    nc.tensor.matmul(psO[:, :P + 1],
                                     lhsT=ex[:, P * kb:P * (kb + 1)],
                                     rhs=vsb[b][:, kb, lh, 0:P + 1],
                                     start=(kb == 0), stop=(kb == qt))
                rinv = p_ln.tile([P, 1], F32, tag="rinv", name=f"ri_{mt}_{lh}")
                nc.vector.reciprocal(rinv[:], psO[:, P:P + 1])
                nc.vector.tensor_scalar_mul(aosb[mt][:, lh, :], psO[:, 0:P], rinv[:])
                if lh == HL - 1:
                    nc.sync.dma_start(aot_dram[P * mt:P * (mt + 1), :],
                                      aosb[mt][:].rearrange("p a b -> p (a b)"))
                    del aosb[mt]

            rg = [list(range(NCORES))]

            def emit_collective(j):
                nc.gpsimd.collective_compute(
                    "AllToAll", mybir.AluOpType.bypass, replica_groups=rg,
                    ins=[aot_dram[1024 * j:1024 * (j + 1), :]],
                    outs=[a2a_dram[1024 * j:1024 * (j + 1), :]])

            # ================= Stage C: O-proj + LN2 per chunk ================
            h2T = p_ali.tile([P, KS, NCHUNK * P], BF16, tag="ali16", name="h2T")

            def emit_C(j):
                af = p_af.tile([P, H], BF16, tag="af", name=f"af_{j}")
                nc.sync.dma_start(
                    af[:].rearrange("p (s f) -> p s f", f=DV),
                    a2a_dram[1024 * j:1024 * (j + 1), :]
                    .rearrange("(s p) f -> p s f", p=P))
                # transpose attn_full -> attnT [feat, tok]
                afT = p_af.tile([P, KS, P], BF16, tag="afT", name=f"afT_{j}")
                for fg in range(2):
                    ptp = psA.tile([P, 1024], BF16, tag="psA", name=f"at_{j}_{fg}")
                    for f4 in range(8):
                        f = 8 * fg + f4
                        nc.tensor.transpose(ptp[:, P * f4:P * (f4 + 1)],
                                            af[:, P * f:P * (f + 1)], ident[:])
                    nc.vector.tensor_copy(
                        out=afT[:, 8 * fg:8 * (fg + 1), :],
                        in_=ptp[:].rearrange("p (a b) -> p a b", b=P))
                # O-projection (2 passes of 2 psum banks) + residual
                x2 = p_x2.tile([P, H], F32, tag="x2", name=f"x2_{j}")
                xr = p_x2.tile([P, H], BF16, tag="xr", name=f"xr_{j}")
                nc.sync.dma_start(xr[:], xres_d[P * j:P * (j + 1), :])
                for half in range(2):
                    pss = []
                    for nn in range(2):
                        psn = psA.tile([P, 512], F32, tag="psA",
                                       name=f"op_{j}_{half}_{nn}")
                        pss.append(psn)
                    for kb in range(KS):
                        for nn in range(2):
                            nk = 2 * half + nn
                            nc.tensor.matmul(
                                pss[nn][:], lhsT=afT[:, kb, :],
                                rhs=wo_sb[:, kb, 512 * nk:512 * (nk + 1)],
                                start=(kb == 0), stop=(kb == KS - 1))
                    for nn in range(2):
                        c0 = 1024 * half + 512 * nn
                        nc.vector.tensor_tensor(x2[:, c0:c0 + 512], pss[nn][:],
                                                xr[:, c0:c0 + 512],
                                                mybir.AluOpType.add)
                nc.sync.dma_start(x2_dram[P * j:P * (j + 1), :], x2[:])
                # LN2 -> h2 -> h2T
                nmu, rstd = layer_norm_stats([(x2, 4)], f"ln2_{j}")
                h2 = p_h.tile([P, H], BF16, tag="ht", name=f"h2_{j}")
                for hh in range(2):
                    ln_apply(h2[:, (H // 2) * hh:(H // 2) * (hh + 1)],
                             x2[:, (H // 2) * hh:(H // 2) * (hh + 1)],
                             nmu, rstd, nc.gpsimd)
                for fg in range(KS // 8):
                    ptp = psA.tile([P, 1024], BF16, tag="psA", name=f"h2t_{j}_{fg}")
                    for f4 in range(8):
                        f = 8 * fg + f4
                        nc.tensor.transpose(ptp[:, P * f4:P * (f4 + 1)],
                                            h2[:, P * f:P * (f + 1)], ident[:])
                    nc.vector.tensor_copy(
                        out=h2T[:, 8 * fg:8 * (fg + 1), P * j:P * (j + 1)],
                        in_=ptp[:].rearrange("p (a b) -> p a b", b=P))

            # ================= emission schedule ==============================
            for g in range(NG_PER_B):
                emit_A_group(0, g)

            # attention(b0) interleaved with QKV(b1)
            for qt in range(QT_PER_B):
                exs = [emit_B_S(0, qt, lh) for lh in range(HL)]
                for lh in range(HL):
                    emit_B_AV(0, qt, lh, exs[lh])
                if qt % 2 == 1:
                    emit_A_group(1, qt // 2)
                if qt == 1:
                    nc.scalar.dma_start(out=wo_sb[:], in_=wo_d[:, :, :])
                if qt == 7:
                    emit_collective(0)
            emit_collective(1)

            # attention(b1) interleaved with chunk 0/1 post-processing
            for qt in range(QT_PER_B):
                exs = [emit_B_S(1, qt, lh) for lh in range(HL)]
                for lh in range(HL):
                    emit_B_AV(1, qt, lh, exs[lh])
                if qt == 3:
                    emit_C(0)
                if qt == 11:
                    emit_C(1)
                if qt == 7:
                    emit_collective(2)
            emit_collective(3)
            emit_C(2)
            emit_C(3)

            # ================= Stage D: MLP1 ==================================
            silu_fn = (mybir.ActivationFunctionType.Sigmoid if sim
                       else mybir.ActivationFunctionType.Silu)
            for mm in range(MMT):
                w1t = p_w1.tile([P, KS, P], BF16, tag="w1t", name=f"w1t_{mm}")
                nc.gpsimd.dma_start(out=w1t[:], in_=w1_d[mm, :, :, :])
                ps = psA.tile([P, 512], F32, tag="psA", name=f"u_{mm}")
                for ks in range(KS):
                    nc.tensor.matmul(ps[:],
                                     lhsT=w1t[:, ks, :],
                                     rhs=h2T[:, ks, :],
                                     start=(ks == 0), stop=(ks == KS - 1))
                usl = p_us.tile([P, 512], BF16, tag="usl", name=f"usl_{mm}")
                nc.scalar.activation(usl[:], ps[:], silu_fn,
                                     bias=b1_sb[:, mm:mm + 1])
                nc.sync.dma_start(ut_dram[P * mm:P * (mm + 1), :], usl[:])

            # ================= Stage E: MLP2 ==================================
            for ng in range(2):
                pss = [[None] * 2 for _ in range(4)]
                for jj in range(4):
                    for nn in range(2):
                        pss[jj][nn] = psA.tile([P, 512], F32, tag="psA",
                                               name=f"y_{ng}_{jj}_{nn}")
                x2sls = [[None] * 2 for _ in range(4)]

                def emit_x2sl(ng, jj):
                    for nn in range(2):
                        c0 = 1024 * ng + 512 * nn
                        x2sls[jj][nn] = p_ev.tile([P, 512], F32, tag="x2sl",
                                                  name=f"x2sl_{ng}_{jj}_{nn}",
                                                  bufs=4)
                        nc.scalar.dma_start(
                            out=x2sls[jj][nn][:],
                            in_=x2_dram[P * jj:P * (jj + 1), c0:c0 + 512])

                for mm in range(MMT):
                    utk = p_uk.tile([P, 512], BF16, tag="utk", name=f"utk_{ng}_{mm}")
                    nc.sync.dma_start(utk[:], ut_dram[P * mm:P * (mm + 1), :])
                    w2t = p_w2.tile([P, 1024], BF16, tag="w2t", name=f"w2t_{ng}_{mm}")
                    nc.gpsimd.dma_start(
                        out=w2t[:], in_=w2_d[P * mm:P * (mm + 1),
                                             1024 * ng:1024 * (ng + 1)])
                    for jj in range(4):
                        for nn in range(2):
                            nc.tensor.matmul(
                                pss[jj][nn][:],
                                lhsT=utk[:, P * jj:P * (jj + 1)],
                                rhs=w2t[:, 512 * nn:512 * (nn + 1)],
                                start=(mm == 0), stop=(mm == MMT - 1))
                    # prefetch the residual slices into SBUF before the drain
                    if mm == MMT - 4:
                        emit_x2sl(ng, 0)
                        emit_x2sl(ng, 1)
                # drain: adds on vector, DMAs on the idle scalar queue so the
                # next pass's utk/w2t prefetch flows on sync/gpsimd
                for jj in range(4):
                    if jj == 2:
                        emit_x2sl(ng, 2)
                        emit_x2sl(ng, 3)
                    for nn in range(2):
                        c0 = 1024 * ng + 512 * nn
                        ot = p_ev.tile([P, 512], F32, tag="ot",
                                       name=f"ot_{ng}_{jj}_{nn}")
                        nc.vector.tensor_tensor(ot[:], pss[jj][nn][:],
                                                x2sls[jj][nn][:],
                                                mybir.AluOpType.add)
                        nc.scalar.dma_start(
                            out=out_d[P * jj:P * (jj + 1), c0:c0 + 512], in_=ot[:])
    nc.compile()
    return nc


def _bf16(a):
    return np.asarray(a, dtype=np.float32).astype(ml_dtypes.bfloat16)


def make_in_maps(x, Wq, Wk, Wv, Wo, g1, bn1, g2, bn2, W1, b1, W2, b2):
    x = np.asarray(x, np.float32)
    x_flat = np.ascontiguousarray(x.reshape(NTOK, H))
    s = np.float32(1.0 / np.sqrt(P))
    fp8_np = mybir.dt.np(FP8)

    wq_eff = (g1[:, None] * np.asarray(Wq, np.float32)) * s
    wk_eff = g1[:, None] * np.asarray(Wk, np.float32)
    wv_eff = g1[:, None] * np.asarray(Wv, np.float32)
    bq = (bn1 @ np.asarray(Wq, np.float32)) * s
    bk = bn1 @ np.asarray(Wk, np.float32)
    bv = bn1 @ np.asarray(Wv, np.float32)
    w1_eff = g2[:, None] * np.asarray(W1, np.float32)
    b1_eff = np.asarray(b1, np.float32) + bn2 @ np.asarray(W1, np.float32)

    # shared tensors
    xbf = np.ascontiguousarray(_bf16(x_flat))
    w1_t = np.ascontiguousarray(
        _bf16(w1_eff).reshape(KS, P, MMT, P).transpose(2, 1, 0, 3))  # [mm, p, ks, mw]
    w2_t = np.ascontiguousarray(_bf16(W2))
    b1m = np.ascontiguousarray(b1_eff.reshape(MMT, P).T.astype(np.float32))
    wo8 = np.ascontiguousarray(
        np.asarray(Wo, np.float32).astype(fp8_np).reshape(KS, P, H).transpose(1, 0, 2))
    ii, jj_ = np.meshgrid(np.arange(P), np.arange(P), indexing="ij")
    cmaskT = np.where(ii <= jj_, 0.0, NEG).astype(np.float32)
    b2f = np.asarray(b2, np.float32)

    in_maps = []
    for c in range(NCORES):
        cs = slice(DV * c, DV * (c + 1))
        wqk = np.concatenate([wq_eff[:, cs], wk_eff[:, cs]], axis=1)  # [H, 512]
        wqk_t = np.ascontiguousarray(
            _bf16(wqk).reshape(KS, P, DQK).transpose(1, 0, 2))
        bqk = np.concatenate([bq[cs], bk[cs]]).astype(np.float32)
        bqk_m = np.ascontiguousarray(bqk.reshape(DQK // P, P).T)
        wv_t = np.ascontiguousarray(
            _bf16(wv_eff[:, cs]).reshape(KS, P, DV).transpose(1, 0, 2))
        bvbc = np.ascontiguousarray(np.broadcast_to(
            bv[cs].astype(np.float32).reshape(1, HL, P), (P, HL, P)))
        xres = np.concatenate(
            [x_flat[1024 * j + P * c:1024 * j + P * (c + 1)] for j in range(NCHUNK)],
            axis=0) + b2f
        in_maps.append({
            "xbf": xbf, "xres": np.ascontiguousarray(_bf16(xres)),
            "wqk": wqk_t, "bqk": bqk_m, "wv": wv_t, "bvbc": bvbc, "wo": wo8,
            "w1": w1_t, "b1": b1m, "w2": w2_t, "cmaskT": cmaskT,
        })
    return in_maps


_NC_CACHE = {}


def kernel(**inputs):
    if "nc" not in _NC_CACHE:
        _NC_CACHE["nc"] = build()
    nc = _NC_CACHE["nc"]
    in_maps = make_in_maps(
        inputs["x"], inputs["Wq"], inputs["Wk"], inputs["Wv"], inputs["Wo"],
        np.asarray(inputs["g1"], np.float32), np.asarray(inputs["bn1"], np.float32),
        np.asarray(inputs["g2"], np.float32), np.asarray(inputs["bn2"], np.float32),
        inputs["W1"], inputs["b1"], inputs["W2"], inputs["b2"])
    res = run_bass_kernel_spmd(nc, in_maps, list(range(NCORES)))
    out = np.empty((NTOK, H), np.float32)
    for c in range(NCORES):
        oc = res.results[c]["out"]
        for j in range(NCHUNK):
            out[1024 * j + P * c:1024 * j + P * (c + 1)] = oc[P * j:P * (j + 1)]
    return out.reshape(B, T, H)


# revision 36
# speedup vs baseline: 1.0789x; 1.0022x over previous
"""Fused transformer block (LN -> causal MHA -> residual -> LN -> SiLU MLP -> residual)
on 8 Trainium2 NeuronCores.

v2 design:
- Tensor-parallel over heads (2 heads/core) for QKV + attention.
- Attention scores computed TRANSPOSED (S^T[k,q]) so the post-softmax matrix is
  already in lhsT layout for the A@V matmul (no PE transposes of probabilities).
  Softmax denominator rides along as a ones-column appended to V.
- AllToAll of the raw per-head attention outputs (2MB) replaces a ReduceScatter
  of partial O-projections (16MB). O-projection happens after the exchange,
  token-local, with the full Wo resident in SBUF as fp8 (weights-only
  quantization; activations stay bf16).
- Token-parallel MLP (512 tokens/core, replicated weights). W1 and W2 are each
  streamed from HBM exactly once. U spills through DRAM (bf16).
- x streamed in bf16 for LN1; residual path f32. b2 folded into the residual on
  the host. LayerNorm affine params folded into adjacent projections.
- Emission order interleaves QKV(batch1) into attention(batch0) and the
  O-proj/LN2 chunk work into attention(batch1) to keep the PE fed.
"""

import sys
import os

for _p in ("/opt/trn_rl_repo", "/root/.axon_site/_ro/trn_rl_repo"):
    if os.path.isdir(_p) and _p not in sys.path:
        sys.path.insert(0, _p)
        break

import numpy as np
import ml_dtypes

import concourse.bass as bass
from concourse import bacc
import concourse.mybir as mybir
import concourse.tile as tile
from concourse.masks import make_identity
from concourse.bass_utils import run_bass_kernel_spmd

F32 = mybir.dt.float32
BF16 = mybir.dt.bfloat16
FP8 = mybir.dt.float8e4


def _install_act_table_hint():
    """Steer the act-table-set chooser so Exp and Ln resolve to the one set
    that contains BOTH (natural_log_exp_and_others). Entry order/count is
    preserved (set ids must still match act_info.json); we only hide Exp/Ln
    from the other sets so the chooser can't split them."""
    import concourse.bacc as _bacc
    if getattr(_bacc, "_act_hint_installed", False):
        return
    _orig = _bacc.get_activation_tables

    def _patched(arch):
        tabs = _orig(arch)
        exp = mybir.ActivationFunctionType.Exp
        ln = mybir.ActivationFunctionType.Ln
        for name, fns in tabs.items():
            if name != "natural_log_exp_and_others":
                fns.discard(exp)
                fns.discard(ln)
        return tabs

    _bacc.get_activation_tables = _patched
    _bacc._act_hint_installed = True


_install_act_table_hint()

P = 128          # partitions / head_dim / token tile
H = 2048         # hidden
KS = H // P      # 16 k-subtiles over hidden
HEADS = 16
HL = 2           # heads per core
NCORES = 8
B = 2
T = 2048
NTOK = B * T     # 4096
TPB = T          # tokens per batch
MID = 4 * H      # 8192
MMT = MID // P   # 64 m-tiles over mid dim
DQK = 2 * HL * P   # 512 rows of fused QK projection per core
DV = HL * P        # 256 V/attention-out features per core
EPS = 1e-5
NEG = -1.0e30

QT_PER_B = TPB // P   # 16 q tiles per batch
MT = NTOK // P        # 32 token m-tiles
NCHUNK = 4            # a2a chunks (1024 tokens each)
GT = 256              # tokens per A-group
NG_PER_B = TPB // GT  # 8 A-groups per batch


def build(sim=False, trn_kwargs=None, trace_sim=False):
    nc = bacc.Bacc(None, num_devices=NCORES, **(trn_kwargs or {}))

    x_d = nc.declare_dram_parameter("xbf", [NTOK, H], BF16, isOutput=False)
    xres_d = nc.declare_dram_parameter("xres", [NCHUNK * P, H], BF16, isOutput=False)
    wqk_d = nc.declare_dram_parameter("wqk", [P, KS, DQK], BF16, isOutput=False)
    bqk_d = nc.declare_dram_parameter("bqk", [P, DQK // P], F32, isOutput=False)
    wv_d = nc.declare_dram_parameter("wv", [P, KS, DV], BF16, isOutput=False)
    bvbc_d = nc.declare_dram_parameter("bvbc", [P, HL, P], F32, isOutput=False)
    wo_d = nc.declare_dram_parameter("wo", [P, KS, H], FP8, isOutput=False)
    w1_d = nc.declare_dram_parameter("w1", [MMT, P, KS, P], BF16, isOutput=False)
    b1_d = nc.declare_dram_parameter("b1", [P, MMT], F32, isOutput=False)
    w2_d = nc.declare_dram_parameter("w2", [MID, H], BF16, isOutput=False)
    cmaskT_d = nc.declare_dram_parameter("cmaskT", [P, P], F32, isOutput=False)
    out_d = nc.declare_dram_parameter("out", [NCHUNK * P, H], F32, isOutput=True)

    from contextlib import ExitStack
    with tile.TileContext(nc, trace_sim=trace_sim) as tc:
        with ExitStack() as stack:
            dram = stack.enter_context(tc.tile_pool(name="dram", bufs=1, space="DRAM"))
            const = stack.enter_context(tc.tile_pool(name="const", bufs=1))
            wbig = stack.enter_context(tc.tile_pool(name="wbig", bufs=1))
            # wqk (16KB/part, dead after QKV) aliases h2T (16KB, live from C on)
            p_ali = stack.enter_context(tc.tile_pool(name="ali16", bufs=1))
            p_x = stack.enter_context(tc.tile_pool(name="xin", bufs=2))
            p_ln = stack.enter_context(tc.tile_pool(name="lnsmall", bufs=3))
            p_h = stack.enter_context(tc.tile_pool(name="htok", bufs=2))
            p_hT = stack.enter_context(tc.tile_pool(name="hT", bufs=2))
            p_kv = stack.enter_context(tc.tile_pool(name="kvq", bufs=2))
            p_ex = stack.enter_context(tc.tile_pool(name="expT", bufs=2))
            p_ao = stack.enter_context(tc.tile_pool(name="aot", bufs=2))
            p_af = stack.enter_context(tc.tile_pool(name="attnf", bufs=1))
            p_x2 = stack.enter_context(tc.tile_pool(name="x2", bufs=1))
            p_w1 = stack.enter_context(tc.tile_pool(name="w1pool", bufs=3))
            p_w2 = stack.enter_context(tc.tile_pool(name="w2pool", bufs=4))
            p_us = stack.enter_context(tc.tile_pool(name="ustage", bufs=2))
            p_uk = stack.enter_context(tc.tile_pool(name="ukpool", bufs=4))
            p_ev = stack.enter_context(tc.tile_pool(name="evict", bufs=2))
            psA = stack.enter_context(tc.tile_pool(name="psA", bufs=8, space="PSUM"))

            # ---- internal DRAM ----
            aot_dram = dram.tile([NTOK, DV], BF16)
            a2a_dram = dram.tile([NTOK, DV], BF16)
            ut_dram = dram.tile([MID, NCHUNK * P], BF16)
            x2_dram = dram.tile([NCHUNK * P, H], F32)

            # ---- constants / weights in SBUF ----
            ident = const.tile([P, P], BF16)
            make_identity(nc, ident)
            epsb = const.tile([P, 1], F32)
            nc.vector.memset(epsb[:], EPS)
            cmaskT = const.tile([P, P], F32)
            nc.sync.dma_start(cmaskT[:], cmaskT_d[:, :])
            bqk_sb = const.tile([P, DQK // P], F32)
            nc.sync.dma_start(bqk_sb[:], bqk_d[:, :])
            bvbc_sb = const.tile([P, HL, P], F32)
            nc.sync.dma_start(bvbc_sb[:], bvbc_d[:, :, :])
            b1_sb = const.tile([P, MMT], F32)
            nc.sync.dma_start(b1_sb[:], b1_d[:, :])
            wqk_sb = p_ali.tile([P, KS, DQK], BF16, tag="ali16", name="wqk_sb")
            # split across queues: startup DMA bandwidth is per-ring limited
            nc.gpsimd.dma_start(out=wqk_sb[:, :KS // 2, :],
                                in_=wqk_d[:, :KS // 2, :])
            nc.scalar.dma_start(out=wqk_sb[:, KS // 2:, :],
                                in_=wqk_d[:, KS // 2:, :])
            # wo (4MB) is DMA'd later, during attention(b0)
            wv_sb = wbig.tile([P, KS, DV], BF16)
            nc.scalar.dma_start(out=wv_sb[:], in_=wv_d[:, :, :])
            wo_sb = wbig.tile([P, KS, H], FP8)

            def layer_norm_stats(parts, name):
                """parts: list of (tile, ncols512) SBUF pieces, 4x512 cols total.
                Returns (nmu, rstd) [P,1] f32 APs; nmu = -mean.
                rstd = exp(-0.5*ln(var+eps)): Ln and Exp live in the SAME
                activation table set, so this never forces a table reload
                against the attention Exps (unlike Sqrt)."""
                st = p_ln.tile([P, 4, 6], F32, tag="lnst", name=f"st_{name}")
                a = 0
                for tile_, n in parts:
                    for i in range(n):
                        nc.vector.bn_stats(st[:, a, :], tile_[:, 512 * i:512 * (i + 1)])
                        a += 1
                assert a == 4
                mv = p_ln.tile([P, 2], F32, tag="lnmv", name=f"mv_{name}")
                nc.vector.bn_aggr(mv[:], st[:])
                lv = p_ln.tile([P, 1], F32, tag="lnsd", name=f"lv_{name}")
                nc.scalar.activation(lv[:], mv[:, 1:2],
                                     mybir.ActivationFunctionType.Ln, bias=epsb[:])
                rstd = p_ln.tile([P, 1], F32, tag="lnrstd", name=f"rstd_{name}")
                nc.scalar.activation(rstd[:], lv[:],
                                     mybir.ActivationFunctionType.Exp, scale=-0.5)
                nmu = p_ln.tile([P, 1], F32, tag="lnnmu", name=f"nmu_{name}")
                nc.vector.tensor_scalar_mul(nmu[:], mv[:, 0:1], -1.0)
                return nmu[:], rstd[:]

            def ln_apply(dst, src, nmu, rstd, engine):
                """dst = (src - mean) * rstd via fused tensor_scalar."""
                engine.tensor_scalar(dst, src, nmu, rstd,
                                     mybir.AluOpType.add, mybir.AluOpType.mult)

            # ================= Stage A: LN1, transpose, QKV ===================
            ksb = [None, None]
            vsb = [None, None]
            qT = [None, None]

            def emit_A_group(b, g):
                """LN1 + transpose + QKV for GT=256 tokens (group g of batch b)."""
                if g == 0:
                    ksb[b] = p_kv.tile([P, HL, TPB], BF16, tag="ksb", name=f"ksb_{b}")
                    vsb[b] = p_kv.tile([P, QT_PER_B, HL, P + 2], BF16, tag="vsb",
                                       name=f"vsb_{b}")
                    qT[b] = p_kv.tile([P, HL, TPB], BF16, tag="qT", name=f"qT_{b}")
                    # ones columns for the softmax-denominator trick
                    nc.vector.memset(vsb[b][:, :, :, P:P + 1], 1.0)
                hT = p_hT.tile([P, KS, GT], BF16, tag="hT", name=f"hT_{b}_{g}")
                # pre-B window: vector is hot (stats), scalar idle -> psum
                # readers on scalar. B window: scalar owns Exp -> use vector.
                if b == 0:
                    ev_copy = lambda out, in_: nc.scalar.copy(out=out, in_=in_)
                    ev_bias = lambda out, in_, s: nc.scalar.add(out, in_, s)
                else:
                    ev_copy = lambda out, in_: nc.vector.tensor_copy(out=out, in_=in_)
                    ev_bias = lambda out, in_, s: nc.vector.tensor_scalar_add(
                        out, in_, s)
                for tt in range(GT // P):   # 128-token LN tiles
                    t = (TPB * b + GT * g) // P + tt
                    xh = []
                    for hh in range(2):
                        xth = p_x.tile([P, H // 2], BF16, tag="xt",
                                       name=f"xt_{t}_{hh}", bufs=4)
                        (nc.sync if hh == 0 else nc.scalar).dma_start(
                            out=xth[:], in_=x_d[P * t:P * (t + 1),
                                               (H // 2) * hh:(H // 2) * (hh + 1)])
                        xh.append(xth)
                    nmu, rstd = layer_norm_stats([(xh[0], 2), (xh[1], 2)],
                                                 f"ln1_{t}")
                    ht = p_h.tile([P, H], BF16, tag="ht", name=f"ht_{t}")
                    for hh in range(2):
                        ln_apply(ht[:, (H // 2) * hh:(H // 2) * (hh + 1)],
                                 xh[hh][:], nmu, rstd, nc.gpsimd)
                    for fg in range(KS // 8):
                        ptp = psA.tile([P, 1024], BF16, tag="psA", name=f"trp_{t}_{fg}")
                        for f4 in range(8):
                            f = 8 * fg + f4
                            nc.tensor.transpose(ptp[:, P * f4:P * (f4 + 1)],
                                                ht[:, P * f:P * (f + 1)], ident[:])
                        ev_copy(hT[:, 8 * fg:8 * (fg + 1), P * tt:P * (tt + 1)],
                                ptp[:].rearrange("p (a b) -> p a b", b=P))

                col0 = GT * g
                # QK projection: m 0,1 -> Q head0/1 ; 2,3 -> K head0/1
                for m in range(4):
                    ps = psA.tile([P, GT], F32, tag="psA", name=f"qk_{b}_{g}_{m}")
                    for ks in range(KS):
                        nc.tensor.matmul(ps[:], lhsT=wqk_sb[:, ks, P * m:P * (m + 1)],
                                         rhs=hT[:, ks, :],
                                         start=(ks == 0), stop=(ks == KS - 1))
                    dst = qT[b] if m < 2 else ksb[b]
                    ev_bias(dst[:, m % 2, col0:col0 + GT], ps[:],
                            bqk_sb[:, m:m + 1])
                # V projection (token-major)
                for m in range(GT // P):
                    ps = psA.tile([P, 512], F32, tag="psA", name=f"v_{b}_{g}_{m}")
                    for ks in range(KS):
                        nc.tensor.matmul(ps[:, :DV], lhsT=hT[:, ks, P * m:P * (m + 1)],
                                         rhs=wv_sb[:, ks, :],
                                         start=(ks == 0), stop=(ks == KS - 1))
                    tm = (GT * g) // P + m
                    nc.vector.tensor_tensor(
                        vsb[b][:, tm, :, 0:P],
                        ps[:, :DV].rearrange("p (a b) -> p a b", b=P),
                        bvbc_sb[:], mybir.AluOpType.add)

            # ================= Stage B: attention (S^T form) ==================
            aosb = {}

            def emit_B_S(b, qt, lh):
                """S^T matmuls + mask + exp for (batch, query tile, local head)."""
                klen = P * (qt + 1)
                nchs = (qt + 4) // 4
                ex = p_ex.tile([P, TPB], BF16, tag="ex", name=f"ex_{b}_{qt}_{lh}")
                qcols = qT[b][:, lh, P * qt:P * (qt + 1)]
                for j in range(nchs):
                    n0 = 512 * j
                    n1 = min(n0 + 512, klen)
                    ps = psA.tile([P, 512], F32, tag="psA", name=f"s_{b}_{qt}_{lh}_{j}")
                    for kb in range(n0 // P, n1 // P):
                        nc.tensor.matmul(ps[:, P * kb - n0:P * (kb + 1) - n0],
                                         lhsT=ksb[b][:, lh, P * kb:P * (kb + 1)],
                                         rhs=qcols, start=True, stop=True)
                    if j == nchs - 1:
                        d0 = klen - P - n0
                        nc.vector.tensor_tensor(ps[:, d0:d0 + P], ps[:, d0:d0 + P],
                                                cmaskT[:], mybir.AluOpType.add)
                    nc.scalar.activation(ex[:, n0:n1], ps[:, :n1 - n0],
                                         mybir.ActivationFunctionType.Exp)
                return ex

            def emit_B_AV(b, qt, lh, ex):
                """A@V with ones-column, normalize, stage aot; DMA after lh=1."""
                mt = QT_PER_B * b + qt
                if lh == 0:
                    aosb[mt] = p_ao.tile([P, HL, P], BF16, tag="aot", name=f"ao_{mt}")
                psO = psA.tile([P, P + 2], F32, tag="psA", name=f"o_{mt}_{lh}")
                for kb in range(qt + 1):
                    nc.tensor.matmul(psO[:, :P + 1],
                                     lhsT=ex[:, P * kb:P * (kb + 1)],
                                     rhs=vsb[b][:, kb, lh, 0:P + 1],
                                     start=(kb == 0), stop=(kb == qt))
                rinv = p_ln.tile([P, 1], F32, tag="rinv", name=f"ri_{mt}_{lh}")
                nc.vector.reciprocal(rinv[:], psO[:, P:P + 1])
                nc.vector.tensor_scalar_mul(aosb[mt][:, lh, :], psO[:, 0:P], rinv[:])
                if lh == HL - 1:
                    nc.sync.dma_start(aot_dram[P * mt:P * (mt + 1), :],
                                      aosb[mt][:].rearrange("p a b -> p (a b)"))
                    del aosb[mt]

            rg = [list(range(NCORES))]

            def emit_collective(j):
                nc.gpsimd.collective_compute(
                    "AllToAll", mybir.AluOpType.bypass, replica_groups=rg,
                    ins=[aot_dram[1024 * j:1024 * (j + 1), :]],
                    outs=[a2a_dram[1024 * j:1024 * (j + 1), :]])

            # ================= Stage C: O-proj + LN2 per chunk ================
            h2T = p_ali.tile([P, KS, NCHUNK * P], BF16, tag="ali16", name="h2T")

            def emit_C(j):
                af = p_af.tile([P, H], BF16, tag="af", name=f"af_{j}")
                nc.sync.dma_start(
                    af[:].rearrange("p (s f) -> p s f", f=DV),
                    a2a_dram[1024 * j:1024 * (j + 1), :]
                    .rearrange("(s p) f -> p s f", p=P))
                # transpose attn_full -> attnT [feat, tok]
                afT = p_af.tile([P, KS, P], BF16, tag="afT", name=f"afT_{j}")
                for fg in range(2):
                    ptp = psA.tile([P, 1024], BF16, tag="psA", name=f"at_{j}_{fg}")
                    for f4 in range(8):
                        f = 8 * fg + f4
                        nc.tensor.transpose(ptp[:, P * f4:P * (f4 + 1)],
                                            af[:, P * f:P * (f + 1)], ident[:])
                    nc.vector.tensor_copy(
                        out=afT[:, 8 * fg:8 * (fg + 1), :],
                        in_=ptp[:].rearrange("p (a b) -> p a b", b=P))
                # O-projection (2 passes of 2 psum banks) + residual
                x2 = p_x2.tile([P, H], F32, tag="x2", name=f"x2_{j}")
                xr = p_x2.tile([P, H], BF16, tag="xr", name=f"xr_{j}")
                nc.sync.dma_start(xr[:], xres_d[P * j:P * (j + 1), :])
                for half in range(2):
                    pss = []
                    for nn in range(2):
                        psn = psA.tile([P, 512], F32, tag="psA",
                                       name=f"op_{j}_{half}_{nn}")
                        pss.append(psn)
                    for kb in range(KS):
                        for nn in range(2):
                            nk = 2 * half + nn
                            nc.tensor.matmul(
                                pss[nn][:], lhsT=afT[:, kb, :],
                                rhs=wo_sb[:, kb, 512 * nk:512 * (nk + 1)],
                                start=(kb == 0), stop=(kb == KS - 1))
                    for nn in range(2):
                        c0 = 1024 * half + 512 * nn
                        nc.vector.tensor_tensor(x2[:, c0:c0 + 512], pss[nn][:],
                                                xr[:, c0:c0 + 512],
                                                mybir.AluOpType.add)
                nc.sync.dma_start(x2_dram[P * j:P * (j + 1), :], x2[:])
                # LN2 -> h2 -> h2T
                nmu, rstd = layer_norm_stats([(x2, 4)], f"ln2_{j}")
                h2 = p_h.tile([P, H], BF16, tag="ht", name=f"h2_{j}")
                for hh in range(2):
                    ln_apply(h2[:, (H // 2) * hh:(H // 2) * (hh + 1)],
                             x2[:, (H // 2) * hh:(H // 2) * (hh + 1)],
                             nmu, rstd, nc.gpsimd)
                for fg in range(KS // 8):
                    ptp = psA.tile([P, 1024], BF16, tag="psA", name=f"h2t_{j}_{fg}")
                    for f4 in range(8):
                        f = 8 * fg + f4
                        nc.tensor.transpose(ptp[:, P * f4:P * (f4 + 1)],
                                            h2[:, P * f:P * (f + 1)], ident[:])
                    nc.vector.tensor_copy(
                        out=h2T[:, 8 * fg:8 * (fg + 1), P * j:P * (j + 1)],
                        in_=ptp[:].rearrange("p (a b) -> p a b", b=P))

            # ================= emission schedule ==============================
            for g in range(NG_PER_B):
                emit_A_group(0, g)

            # attention(b0) interleaved with QKV(b1)
            for qt in range(QT_PER_B):
                exs = [emit_B_S(0, qt, lh) for lh in range(HL)]
                for lh in range(HL):
                    emit_B_AV(0, qt, lh, exs[lh])
                if qt % 2 == 1:
                    emit_A_group(1, qt // 2)
                if qt == 1:
                    nc.scalar.dma_start(out=wo_sb[:], in_=wo_d[:, :, :])
                if qt == 7:
                    emit_collective(0)
            emit_collective(1)

            # attention(b1) interleaved with chunk 0/1 post-processing
            for qt in range(QT_PER_B):
                exs = [emit_B_S(1, qt, lh) for lh in range(HL)]
                for lh in range(HL):
                    emit_B_AV(1, qt, lh, exs[lh])
                if qt == 3:
                    emit_C(0)
                if qt == 11:
                    emit_C(1)
                if qt == 7:
                    emit_collective(2)
            emit_collective(3)
            emit_C(2)
            emit_C(3)

            # ================= Stage D: MLP1 ==================================
            silu_fn = (mybir.ActivationFunctionType.Sigmoid if sim
                       else mybir.ActivationFunctionType.Silu)
            for mm in range(MMT):
                w1t = p_w1.tile([P, KS, P], BF16, tag="w1t", name=f"w1t_{mm}")
                nc.gpsimd.dma_start(out=w1t[:], in_=w1_d[mm, :, :, :])
                ps = psA.tile([P, 512], F32, tag="psA", name=f"u_{mm}")
                for ks in range(KS):
                    nc.tensor.matmul(ps[:],
                                     lhsT=w1t[:, ks, :],
                                     rhs=h2T[:, ks, :],
                                     start=(ks == 0), stop=(ks == KS - 1))
                usl = p_us.tile([P, 512], BF16, tag="usl", name=f"usl_{mm}")
                nc.scalar.activation(usl[:], ps[:], silu_fn,
                                     bias=b1_sb[:, mm:mm + 1])
                nc.sync.dma_start(ut_dram[P * mm:P * (mm + 1), :], usl[:])

            # ================= Stage E: MLP2 ==================================
            for ng in range(2):
                pss = [[None] * 2 for _ in range(4)]
                for jj in range(4):
                    for nn in range(2):
                        pss[jj][nn] = psA.tile([P, 512], F32, tag="psA",
                                               name=f"y_{ng}_{jj}_{nn}")
                x2sls = [[None] * 2 for _ in range(4)]

                def emit_x2sl(ng, jj):
                    for nn in range(2):
                        c0 = 1024 * ng + 512 * nn
                        x2sls[jj][nn] = p_ev.tile([P, 512], F32, tag="x2sl",
                                                  name=f"x2sl_{ng}_{jj}_{nn}",
                                                  bufs=4)
                        nc.scalar.dma_start(
                            out=x2sls[jj][nn][:],
                            in_=x2_dram[P * jj:P * (jj + 1), c0:c0 + 512])

                for mm in range(MMT):
                    utk = p_uk.tile([P, 512], BF16, tag="utk", name=f"utk_{ng}_{mm}")
                    nc.sync.dma_start(utk[:], ut_dram[P * mm:P * (mm + 1), :])
                    w2t = p_w2.tile([P, 1024], BF16, tag="w2t", name=f"w2t_{ng}_{mm}")
                    nc.gpsimd.dma_start(
                        out=w2t[:], in_=w2_d[P * mm:P * (mm + 1),
                                             1024 * ng:1024 * (ng + 1)])
                    for jj in range(4):
                        for nn in range(2):
                            nc.tensor.matmul(
                                pss[jj][nn][:],
                                lhsT=utk[:, P * jj:P * (jj + 1)],
                                rhs=w2t[:, 512 * nn:512 * (nn + 1)],
                                start=(mm == 0), stop=(mm == MMT - 1))
                    # prefetch the residual slices into SBUF before the drain
                    if mm == MMT - 4:
                        emit_x2sl(ng, 0)
                        emit_x2sl(ng, 1)
                # drain: adds on vector, DMAs on the idle scalar queue so the
                # next pass's utk/w2t prefetch flows on sync/gpsimd
                for jj in range(4):
                    if jj == 2:
                        emit_x2sl(ng, 2)
                        emit_x2sl(ng, 3)
                    for nn in range(2):
                        c0 = 1024 * ng + 512 * nn
                        ot = p_ev.tile([P, 512], F32, tag="ot",
                                       name=f"ot_{ng}_{jj}_{nn}")
                        nc.vector.tensor_tensor(ot[:], pss[jj][nn][:],
                                                x2sls[jj][nn][:],
                                                mybir.AluOpType.add)
                        nc.scalar.dma_start(
                            out=out_d[P * jj:P * (jj + 1), c0:c0 + 512], in_=ot[:])
    nc.compile()
    return nc


def _bf16(a):
    return np.asarray(a, dtype=np.float32).astype(ml_dtypes.bfloat16)


def make_in_maps(x, Wq, Wk, Wv, Wo, g1, bn1, g2, bn2, W1, b1, W2, b2):
    x = np.asarray(x, np.float32)
    x_flat = np.ascontiguousarray(x.reshape(NTOK, H))
    s = np.float32(1.0 / np.sqrt(P))
    fp8_np = mybir.dt.np(FP8)

    wq_eff = (g1[:, None] * np.asarray(Wq, np.float32)) * s
    wk_eff = g1[:, None] * np.asarray(Wk, np.float32)
    wv_eff = g1[:, None] * np.asarray(Wv, np.float32)
    bq = (bn1 @ np.asarray(Wq, np.float32)) * s
    bk = bn1 @ np.asarray(Wk, np.float32)
    bv = bn1 @ np.asarray(Wv, np.float32)
    w1_eff = g2[:, None] * np.asarray(W1, np.float32)
    b1_eff = np.asarray(b1, np.float32) + bn2 @ np.asarray(W1, np.float32)

    # shared tensors
    xbf = np.ascontiguousarray(_bf16(x_flat))
    w1_t = np.ascontiguousarray(
        _bf16(w1_eff).reshape(KS, P, MMT, P).transpose(2, 1, 0, 3))  # [mm, p, ks, mw]
    w2_t = np.ascontiguousarray(_bf16(W2))
    b1m = np.ascontiguousarray(b1_eff.reshape(MMT, P).T.astype(np.float32))
    wo8 = np.ascontiguousarray(
        np.asarray(Wo, np.float32).astype(fp8_np).reshape(KS, P, H).transpose(1, 0, 2))
    ii, jj_ = np.meshgrid(np.arange(P), np.arange(P), indexing="ij")
    cmaskT = np.where(ii <= jj_, 0.0, NEG).astype(np.float32)
    b2f = np.asarray(b2, np.float32)

    in_maps = []
    for c in range(NCORES):
        cs = slice(DV * c, DV * (c + 1))
        wqk = np.concatenate([wq_eff[:, cs], wk_eff[:, cs]], axis=1)  # [H, 512]
        wqk_t = np.ascontiguousarray(
            _bf16(wqk).reshape(KS, P, DQK).transpose(1, 0, 2))
        bqk = np.concatenate([bq[cs], bk[cs]]).astype(np.float32)
        bqk_m = np.ascontiguousarray(bqk.reshape(DQK // P, P).T)
        wv_t = np.ascontiguousarray(
            _bf16(wv_eff[:, cs]).reshape(KS, P, DV).transpose(1, 0, 2))
        bvbc = np.ascontiguousarray(np.broadcast_to(
            bv[cs].astype(np.float32).reshape(1, HL, P), (P, HL, P)))
        xres = np.concatenate(
            [x_flat[1024 * j + P * c:1024 * j + P * (c + 1)] for j in range(NCHUNK)],
            axis=0) + b2f
        in_maps.append({
            "xbf": xbf, "xres": np.ascontiguousarray(_bf16(xres)),
            "wqk": wqk_t, "bqk": bqk_m, "wv": wv_t, "bvbc": bvbc, "wo": wo8,
            "w1": w1_t, "b1": b1m, "w2": w2_t, "cmaskT": cmaskT,
        })
    return in_maps


_NC_CACHE = {}


def kernel(**inputs):
    if "nc" not in _NC_CACHE:
        _NC_CACHE["nc"] = build()
    nc = _NC_CACHE["nc"]
    in_maps = make_in_maps(
        inputs["x"], inputs["Wq"], inputs["Wk"], inputs["Wv"], inputs["Wo"],
        np.asarray(inputs["g1"], np.float32), np.asarray(inputs["bn1"], np.float32),
        np.asarray(inputs["g2"], np.float32), np.asarray(inputs["bn2"], np.float32),
        inputs["W1"], inputs["b1"], inputs["W2"], inputs["b2"])
    res = run_bass_kernel_spmd(nc, in_maps, list(range(NCORES)))
    out = np.empty((NTOK, H), np.float32)
    for c in range(NCORES):
        oc = res.results[c]["out"]
        for j in range(NCHUNK):
            out[1024 * j + P * c:1024 * j + P * (c + 1)] = oc[P * j:P * (j + 1)]
    return out.reshape(B, T, H)
